# revision 28
# baseline (speedup 1.0000x reference)
"""Trainium2 Bass kernel for nn_AttentionBlock (GroupNorm + single-head self-attention + proj + residual).

Input  x [4, 512, 64, 64] f32.  8 NeuronCores: core i handles batch b=i//2,
query-half h=i%2 (2048 of the 4096 spatial positions).  Each core computes
GroupNorm + full K/V for its batch element, Q only for its half, flash-style
attention over all 4096 keys, the output projection and residual for its half.
No collectives: host shards inputs / gathers outputs.

Layouts (per core), n=4096, nq=2048, c=512:
  hf  = GN(x)      [c, n]    (4 SBUF tiles [128, 4096], in-place over x)
  k   = Wk hf + bk [c, n]    (resident SBUF, 4 tiles [128, 4096])
  q   = Wq hf + bq [c, nq]   (spilled to DRAM, streamed back per q-chunk)
  v^T = (Wv hf + bv)^T [n, c] (spilled to DRAM, streamed back per (qc, j))
  S^T chunk [j:128, q:512] = k_chunk^T q_chunk   (PSUM)
  E = exp(S^T * 1/sqrt(c))  -> O[co,q] += v^T_j[:,co]^T E ;  denom[1,q] += 1^T E
  y^T [q:128, c:512] = (O chunk)^T Wp^T ; y = y^T * (1/denom) + (x^T + b_proj)
"""

import os
import numpy as np

B, C, HH, WW = 4, 512, 64, 64
N = HH * WW            # 4096
NQ = N // 2            # 2048 queries per core
NCORES = 8
CT = C // 128          # 4 channel tiles
PT = N // 512          # 8 spatial chunks of 512
QT = NQ // 512         # 4 query chunks of 512
JT = N // 128          # 32 key chunks of 128
GSIZE = 16             # channels per group
EPS = 1e-5
SCALE = 1.0 / float(np.sqrt(C))

MM_DT_NAME = os.environ.get("KERNEL_MM_DT", "float32r")

_PROG = None
_PROG_DT = None


def _build_program(mm_dt_name):
    import concourse.bacc as bacc
    import concourse.tile as tile
    from concourse import mybir
    from contextlib import ExitStack

    F32 = mybir.dt.float32
    MM = getattr(mybir.dt, mm_dt_name)

    nc = bacc.Bacc("TRN2", target_bir_lowering=False, debug=False,
                   num_devices=NCORES)

    def din(name, shape, dt=None):
        return nc.dram_tensor(name, shape, dt or F32, kind="ExternalInput").ap()

    x_cn = din("x_cn", [C, N], MM)      # x for this batch, query-half first
    xb_t = din("xb_t", [NQ, C])         # x^T residual slice + b_proj
    w_qT = din("w_qT", [C, C], MM)
    w_kT = din("w_kT", [C, C], MM)
    w_vT = din("w_vT", [C, C], MM)
    w_pT = din("w_pT", [C, C], MM)
    b_v = din("b_v", [1, C], MM)
    cols = din("cols", [128, 4 * CT])   # per c-tile: gamma, beta, b_q, b_k
    gmat = din("gmat", [128, 8])        # group membership (p//16 == u)
    gmat_t = din("gmat_t", [8, 128])
    y_t = nc.dram_tensor("y_t", [NQ, C], F32, kind="ExternalOutput").ap()

    AF = mybir.ActivationFunctionType
    OP = mybir.AluOpType

    with tile.TileContext(nc) as tc, ExitStack() as ctx:
        persist = ctx.enter_context(tc.tile_pool(name="persist", bufs=1))
        kpool = ctx.enter_context(tc.tile_pool(name="kpool", bufs=1))
        qa0pool = ctx.enter_context(tc.tile_pool(name="qa0", bufs=1))
        dram = ctx.enter_context(tc.tile_pool(name="dram", bufs=1, space="DRAM"))

        # ---- persistent small constants ----
        gma = persist.tile([128, 8], F32)
        nc.sync.dma_start(out=gma, in_=gmat)
        gmt = persist.tile([8, 128], F32)
        nc.sync.dma_start(out=gmt, in_=gmat_t)
        one1 = persist.tile([1, 1], F32)
        nc.vector.memset(one1, 1.0)
        ones_row = persist.tile([1, 128], MM)
        ones_col = persist.tile([128, 1], MM)
        if MM is F32:
            nc.vector.memset(ones_row, 1.0)
            nc.vector.memset(ones_col, 1.0)
        else:
            ones_st_r = persist.tile([1, 128], F32)
            nc.vector.memset(ones_st_r, 1.0)
            nc.vector.tensor_copy(ones_row, ones_st_r)
            ones_st_c = persist.tile([128, 1], F32)
            nc.vector.memset(ones_st_c, 1.0)
            nc.vector.tensor_copy(ones_col, ones_st_c)
        eps8 = persist.tile([8, 1], F32)
        nc.vector.memset(eps8, EPS)
        warm_a = persist.tile([128, 128], mybir.dt.bfloat16)
        nc.vector.memset(warm_a, 0.03)

        def emit_burst(wpool, wppool, dep_ap, n, nm, pstag="warmps"):
            # Dense bf16 matmuls (normal PE mode) chained to dep_ap: trips the
            # PE activity monitor into the fast-clock state.  fp32r matmuls
            # alone don't reliably do this.
            wb = wpool.tile([128, 512], mybir.dt.bfloat16, tag="warmb",
                            name=f"wb_{nm}")
            nc.vector.tensor_copy(wb, dep_ap)
            wps = wppool.tile([128, 512], F32, tag=pstag,
                              name=f"wps_{nm}", bufs=2)
            for wi in range(n):
                nc.tensor.matmul(wps, warm_a, wb,
                                 start=(wi == 0), stop=(wi == n - 1))
        cols_t = persist.tile([128, 4 * CT], F32)
        nc.sync.dma_start(out=cols_t, in_=cols)
        gcol = [cols_t[:, 4 * t:4 * t + 1] for t in range(CT)]
        bcol = [cols_t[:, 4 * t + 1:4 * t + 2] for t in range(CT)]
        bqcol = [cols_t[:, 4 * t + 2:4 * t + 3] for t in range(CT)]
        bkcol = [cols_t[:, 4 * t + 3:4 * t + 4] for t in range(CT)]
        # b_v row (already MM dtype in DRAM)
        bvr = persist.tile([1, C], MM)
        nc.sync.dma_start(out=bvr, in_=b_v)
        # w_proj^T tiles (persist: used at the end of every q-chunk)
        wp_big = persist.tile([128, CT, C], MM)
        nc.sync.dma_start(out=wp_big,
                          in_=w_pT.rearrange("(t p) o -> p t o", t=CT))
        wp = [wp_big[:, t, :] for t in range(CT)]

        # k stays resident for the whole kernel
        k_tiles = [kpool.tile([128, N], MM, name=f"k_{t}", tag=f"k{t}")
                   for t in range(CT)]
        # first 512 query columns stay resident: attention qc=0 starts without
        # waiting for the q spill/reload round-trip
        qa0_tiles = [qa0pool.tile([128, 512], MM, name=f"qa0_{t}",
                                  tag=f"qa0{t}") for t in range(CT)]
        # spill targets
        vt_dram = dram.tile([JT, 128, C], MM)     # v^T as 32 j-tiles [128, 512]
        q_dram = dram.tile([CT, 128, NQ], MM)     # q in [c, nq] layout

        # GroupNorm is folded into the qkv weights:  hf = x*sc + bc  =>
        #   q/k/v = (W ∘ sc) x + (W bc + b_qkv).
        # x arrives pre-rounded to the matmul dtype, so it feeds the matmuls
        # directly; no hf tiles and no big apply pass.  (Assumes gamma has no
        # exact zeros: the weight-bias matvec uses bc/sc on the scaled W.)
        with tc.tile_pool(name="xpool", bufs=1) as xpool, \
             tc.tile_pool(name="wmat", bufs=1) as wmat, \
             tc.tile_pool(name="qkvsb", bufs=3) as qkvsb, \
             tc.tile_pool(name="gnsb", bufs=2) as gnsb, \
             tc.tile_pool(name="qps", bufs=1, space="PSUM") as qps:

            x_tiles = [xpool.tile([128, N], MM, name=f"x_{t}", tag=f"x{t}")
                       for t in range(CT)]
            from concourse.bass import _add_dep_helper
            prev_dma = None
            for t in range(CT):
                dma_i = nc.sync.dma_start(out=x_tiles[t],
                                          in_=x_cn[t * 128:(t + 1) * 128, :])
                if prev_dma is not None:
                    _add_dep_helper(dma_i.ins, prev_dma.ins, sync=True,
                                    reason="serialize x tile loads")
                prev_dma = dma_i

            def load_w(srcw, nm):
                w_big = wmat.tile([128, CT, C], MM, name=f"{nm}_big",
                                  tag="w", bufs=2)
                nc.sync.dma_start(
                    out=w_big,
                    in_=srcw.rearrange("(t p) o -> p t o", t=CT))
                return [w_big[:, t, :] for t in range(CT)]

            wv = load_w(w_vT, "wv")

            # ---------------- GroupNorm statistics ----------------
            sc_f = []
            bct = []
            wb_init = gnsb.tile([128, 512], mybir.dt.bfloat16, tag="warmb")
            nc.vector.memset(wb_init, 0.01)
            wps_init = qps.tile([128, 512], F32, tag="g", bufs=2)
            for wi in range(25):
                nc.tensor.matmul(wps_init, warm_a, wb_init,
                                 start=(wi == 0), stop=(wi == 24))
            BRIDGE = (35, 25, 20, 15)
            for t in range(CT):
                x_f = (x_tiles[t] if MM is F32 else x_tiles[t].bitcast(F32))
                # bf16 junk matmuls chained to this x tile: keep the PE busy
                # (and its activity monitor in the fast-clock state) while the
                # GroupNorm statistics for later tiles are still in flight
                emit_burst(gnsb, qps, x_tiles[t][:, 0:512], BRIDGE[t],
                           f"br{t}", pstag="g")
                # st2 = [sum(x), sum(x^2)] per channel: plain sum on DVE,
                # squared sum via ACT Square + accumulate (k tile is scratch)
                st2 = gnsb.tile([128, 2], F32, tag="st2")
                nc.vector.reduce_sum(out=st2[:, 0:1], in_=x_f,
                                     axis=mybir.AxisListType.X)
                nc.scalar.activation(out=k_tiles[t], in_=x_f, func=AF.Square,
                                     accum_out=st2[:, 1:2])
                # group-aggregate: [8, 2] sums over the 16 channels per group
                gps = qps.tile([8, 2], F32, tag="g", bufs=2)
                nc.tensor.matmul(gps, gma, st2, start=True, stop=True)
                grp = gnsb.tile([8, 2], F32, tag="grp")
                nc.scalar.mul(out=grp, in_=gps, mul=1.0 / (GSIZE * N))
                gm2 = gnsb.tile([8, 1], F32, tag="gm2")
                nc.vector.tensor_mul(gm2, grp[:, 0:1], grp[:, 0:1])
                var = gnsb.tile([8, 1], F32, tag="var")
                nc.vector.tensor_sub(var, grp[:, 1:2], gm2)
                std = gnsb.tile([8, 1], F32, tag="std")
                nc.scalar.activation(out=std, in_=var, func=AF.Sqrt,
                                     bias=eps8, scale=1.0)
                gout = gnsb.tile([8, 2], F32, tag="gout")
                nc.vector.tensor_copy(gout[:, 0:1], grp[:, 0:1])
                nc.vector.reciprocal(out=gout[:, 1:2], in_=std)
                # expand group stats back to per-channel [128, 2]
                eps_ps = qps.tile([128, 2], F32, tag="g", bufs=2,
                                  name=f"eps_ps{t}")
                nc.tensor.matmul(eps_ps, gmt, gout, start=True, stop=True)
                pg = gnsb.tile([128, 2], F32, tag="pg")
                nc.scalar.copy(out=pg, in_=eps_ps)
                # per-channel scale = gamma*rstd ; bias = beta - mean*scale
                sc_t = gnsb.tile([128, 1], F32, tag=f"sc{t}", bufs=1)
                nc.vector.tensor_mul(sc_t, gcol[t], pg[:, 1:2])
                sc_f.append(sc_t)
                bc_t = gnsb.tile([128, 1], F32, tag="bc")
                nc.vector.tensor_mul(bc_t, pg[:, 0:1], sc_t)
                nc.vector.tensor_sub(bc_t, bcol[t], bc_t)
                rsc = gnsb.tile([128, 1], F32, tag="rsc")
                nc.vector.reciprocal(out=rsc, in_=sc_t)
                bct_t = gnsb.tile([128, 1], MM, tag=f"bct{t}", bufs=1)
                nc.vector.tensor_mul(bct_t, bc_t, rsc)
                bct.append(bct_t)
                # scale this channel-tile of Wv in place (Wk/Wq later)
                wv_f = (wv[t] if MM is F32 else wv[t].bitcast(F32))
                nc.vector.tensor_scalar_mul(out=wv[t], in0=wv_f,
                                            scalar1=sc_t)

            # weight-bias matvecs:  row_m = sum_c (bc/sc)_c^T (W ∘ sc)_c
            def bias_row(tiles, nm):
                row_ps = qps.tile([1, C], F32, tag="g", bufs=2,
                                  name=f"brow_{nm}")
                for c in range(CT):
                    nc.tensor.matmul(row_ps, bct[c], tiles[c],
                                     start=(c == 0), stop=(c == CT - 1))
                row_sb = gnsb.tile([1, C], F32, tag=f"brs_{nm}", bufs=1)
                nc.scalar.copy(out=row_sb, in_=row_ps)
                return row_sb

            def bias_cols(row_sb, host_cols, nm):
                cols = []
                for o in range(CT):
                    bt_ps = qps.tile([128, 1], F32, tag="g", bufs=2,
                                     name=f"bt_{nm}{o}")
                    nc.tensor.transpose(bt_ps,
                                        row_sb[0:1, o * 128:(o + 1) * 128],
                                        one1)
                    tot = gnsb.tile([128, 1], F32, tag=f"btot_{nm}{o}",
                                    bufs=1)
                    nc.vector.tensor_add(tot, bt_ps, host_cols[o])
                    cols.append(tot)
                return cols

            def scale_w(tiles):
                for t in range(CT):
                    m_f = (tiles[t] if MM is F32 else tiles[t].bitcast(F32))
                    nc.vector.tensor_scalar_mul(out=tiles[t], in0=m_f,
                                                scalar1=sc_f[t])

            vrow = bias_row(wv, "v")
            bvr_tot = gnsb.tile([1, C], MM, tag="bvrt", bufs=1)
            nc.vector.tensor_add(bvr_tot, vrow,
                                 (bvr if MM is F32 else bvr.bitcast(F32)))
            # k weights: second buffer slot is free, so this overlaps v-phase
            wk_l = load_w(w_kT, "wk")
            scale_w(wk_l)
            bk_tot = bias_cols(bias_row(wk_l, "k"), bkcol, "k")

            # ---------------- QKV ----------------
            # v^T = x^T Wv' + bv' : 32 tiles [128p, 512c] -> DRAM
            for p in range(JT):
                vt_ps = qps.tile([128, C], F32, tag="mm", bufs=4)
                for c in range(CT):
                    nc.tensor.matmul(vt_ps,
                                     x_tiles[c][:, p * 128:(p + 1) * 128],
                                     wv[c], start=(c == 0), stop=False)
                nc.tensor.matmul(vt_ps, ones_row, bvr_tot,
                                 start=False, stop=True)
                vt_sb = qkvsb.tile([128, C], MM, tag="vt")
                nc.scalar.copy(out=vt_sb, in_=vt_ps)
                nc.sync.dma_start(out=vt_dram[p], in_=vt_sb)
                if p in (10, 21):
                    emit_burst(qkvsb, qps, vt_sb, 15, f"v{p}", pstag="g")

            # q weights reuse Wv's buffer slots (released by the v-phase)
            wq_l = load_w(w_qT, "wq")
            scale_w(wq_l)
            bq_tot = bias_cols(bias_row(wq_l, "q"), bqcol, "q")

            # k = Wk'^T x + bk' : resident [c,n] tiles
            for o in range(CT):
                for p in range(PT):
                    k_ps = qps.tile([128, 512], F32, tag="mm", bufs=4)
                    for c in range(CT):
                        nc.tensor.matmul(k_ps,
                                         wk_l[c][:, o * 128:(o + 1) * 128],
                                         x_tiles[c][:, p * 512:(p + 1) * 512],
                                         start=(c == 0), stop=(c == CT - 1))
                    nc.vector.tensor_scalar_add(
                        out=k_tiles[o][:, p * 512:(p + 1) * 512],
                        in0=k_ps, scalar1=bk_tot[o])
                    if (o, p) in ((0, 0), (2, 0)):
                        emit_burst(qkvsb, qps,
                                   k_tiles[o][:, p * 512:(p + 1) * 512],
                                   15, f"k{o}", pstag="g")

            # q = Wq'^T x + bq' for first NQ columns; p-major so the first
            # 512 query columns are ready first (they stay resident in SBUF)
            for p in range(QT):
                for o in range(CT):
                    q_ps = qps.tile([128, 512], F32, tag="mm", bufs=4)
                    for c in range(CT):
                        nc.tensor.matmul(q_ps,
                                         wq_l[c][:, o * 128:(o + 1) * 128],
                                         x_tiles[c][:, p * 512:(p + 1) * 512],
                                         start=(c == 0), stop=(c == CT - 1))
                    if p == 0:
                        nc.vector.tensor_scalar_add(out=qa0_tiles[o],
                                                    in0=q_ps,
                                                    scalar1=bq_tot[o])
                        if o == 0:
                            emit_burst(qkvsb, qps, qa0_tiles[0], 15,
                                       "qst", pstag="g")
                    else:
                        q_sb = qkvsb.tile([128, 512], MM, tag="q")
                        nc.vector.tensor_scalar_add(out=q_sb, in0=q_ps,
                                                    scalar1=bq_tot[o])
                        nc.sync.dma_start(
                            out=q_dram[o][:, p * 512:(p + 1) * 512],
                            in_=q_sb)

        # ---------------- attention + proj (per 512-wide q-chunk) ----------------
        with tc.tile_pool(name="qa", bufs=3) as qapool, \
             tc.tile_pool(name="estream", bufs=4) as epool, \
             tc.tile_pool(name="vstream", bufs=6) as vpool, \
             tc.tile_pool(name="osb", bufs=2) as opool, \
             tc.tile_pool(name="ysb", bufs=2) as ypool, \
             tc.tile_pool(name="xbst", bufs=3) as xbpool, \
             tc.tile_pool(name="dsb", bufs=2) as dpool, \
             tc.tile_pool(name="psS", bufs=2, space="PSUM") as psS, \
             tc.tile_pool(name="psO", bufs=1, space="PSUM") as psO, \
             tc.tile_pool(name="psD", bufs=1, space="PSUM") as psD, \
             tc.tile_pool(name="psY", bufs=1, space="PSUM") as psY:

            for qc in range(QT):
                if qc == 0:
                    qa = qa0_tiles
                else:
                    # stream this q-chunk back: qa[c] = q[c, qc*512 : +512]
                    qa = []
                    for c in range(CT):
                        qa_c = qapool.tile([128, 512], MM, name=f"qa_{c}",
                                           tag=f"qa{c}")
                        nc.sync.dma_start(
                            out=qa_c,
                            in_=q_dram[c][:, qc * 512:(qc + 1) * 512])
                        qa.append(qa_c)

                o_ps = [psO.tile([128, 512], F32, name=f"o_ps{co}",
                                 tag=f"o{co}") for co in range(CT)]
                d_ps = psD.tile([1, 512], F32, tag="d")

                def s_exp_v(j):
                    # S^T chunk + exp, and prefetch v^T tile for this j
                    vt_sb = vpool.tile([128, C], MM, tag="vt2")
                    nc.sync.dma_start(out=vt_sb, in_=vt_dram[j])
                    s_ps = psS.tile([128, 512], F32, tag="s")
                    for c in range(CT):
                        nc.tensor.matmul(s_ps,
                                         k_tiles[c][:, j * 128:(j + 1) * 128],
                                         qa[c], start=(c == 0), stop=(c == CT - 1))
                    e_sb = epool.tile([128, 512], MM, tag="e")
                    nc.scalar.activation(out=e_sb, in_=s_ps, func=AF.Exp,
                                         scale=SCALE)
                    return e_sb, vt_sb

                e_cur, v_cur = s_exp_v(0)
                for j in range(JT):
                    nxt = s_exp_v(j + 1) if j + 1 < JT else None
                    first, last = (j == 0), (j == JT - 1)
                    for co in range(CT):
                        nc.tensor.matmul(o_ps[co],
                                         v_cur[:, co * 128:(co + 1) * 128],
                                         e_cur, start=first, stop=last)
                    nc.tensor.matmul(d_ps, ones_col, e_cur,
                                     start=first, stop=last)
                    if nxt is not None:
                        e_cur, v_cur = nxt

                # denominators -> per-query reciprocal [128,1] per 128-row block
                d_sb = dpool.tile([1, 512], F32, tag="dsb")
                nc.scalar.copy(out=d_sb, in_=d_ps)
                rc = []
                for qs in range(4):
                    dt_ps = psD.tile([128, 1], F32, name=f"dt_ps{qs}", tag="d")
                    nc.tensor.transpose(dt_ps,
                                        d_sb[0:1, qs * 128:(qs + 1) * 128],
                                        one1)
                    rc_t = dpool.tile([128, 1], F32, name=f"rc_{qs}",
                                      tag=f"rc{qs}")
                    nc.vector.reciprocal(out=rc_t, in_=dt_ps)
                    rc.append(rc_t)

                # O -> SBUF (rounds to MM dtype)
                o_sb = []
                for co in range(CT):
                    o_t = opool.tile([128, 512], MM, name=f"o_sb{co}",
                                     tag=f"ob{co}")
                    nc.vector.tensor_copy(o_t, o_ps[co])
                    o_sb.append(o_t)

                # proj + 1/denom + residual, per 128-row output block
                for qs in range(4):
                    y_ps = psY.tile([128, C], F32, name=f"y_ps{qs}",
                                    tag="y")
                    for c in range(CT):
                        nc.tensor.matmul(y_ps,
                                         o_sb[c][:, qs * 128:(qs + 1) * 128],
                                         wp[c], start=(c == 0), stop=(c == CT - 1))
                    row0 = qc * 512 + qs * 128
                    xb_sb = xbpool.tile([128, C], F32, tag="xb")
                    nc.sync.dma_start(out=xb_sb, in_=xb_t[row0:row0 + 128, :])
                    y1 = ypool.tile([128, C], F32, tag="y1")
                    nc.vector.tensor_scalar_mul(out=y1, in0=y_ps, scalar1=rc[qs])
                    yo = ypool.tile([128, C], F32, tag="yo")
                    nc.vector.tensor_add(yo, y1, xb_sb)
                    nc.sync.dma_start(out=y_t[row0:row0 + 128, :], in_=yo)

    nc.compile()
    return nc


def _get_prog():
    global _PROG, _PROG_DT
    if _PROG is None or _PROG_DT != MM_DT_NAME:
        _PROG = _build_program(MM_DT_NAME)
        _PROG_DT = MM_DT_NAME
    return _PROG


def _round_f32r(a):
    """RNE to 11 explicit mantissa bits (the fp32r matmul input format)."""
    if MM_DT_NAME != "float32r":
        return a
    b = np.ascontiguousarray(a, dtype=np.float32).view(np.uint32)
    shift = 12
    lsb = (b >> shift) & 1
    mask = np.uint32((~((1 << shift) - 1)) & 0xFFFFFFFF)
    out = (b + np.uint32((1 << (shift - 1)) - 1) + lsb) & mask
    return out.view(np.float32)


def kernel(x, gamma, beta, w_qkv, b_qkv, w_proj, b_proj):
    from concourse.bass_utils import run_bass_kernel_spmd

    x = np.asarray(x, dtype=np.float32)
    gamma = np.asarray(gamma, dtype=np.float32)
    beta = np.asarray(beta, dtype=np.float32)
    w_qkv = np.asarray(w_qkv, dtype=np.float32)
    b_qkv = np.asarray(b_qkv, dtype=np.float32)
    w_proj = np.asarray(w_proj, dtype=np.float32)
    b_proj = np.asarray(b_proj, dtype=np.float32)

    shared = {
        "w_qT": _round_f32r(w_qkv[0:C].T),
        "w_kT": _round_f32r(w_qkv[C:2 * C].T),
        "w_vT": _round_f32r(w_qkv[2 * C:3 * C].T),
        "w_pT": _round_f32r(w_proj.T),
        "b_v": _round_f32r(b_qkv[2 * C:3 * C].reshape(1, C)),
        "cols": np.stack([gamma.reshape(CT, 128),
                          beta.reshape(CT, 128),
                          b_qkv[0:C].reshape(CT, 128),
                          b_qkv[C:2 * C].reshape(CT, 128)],
                         axis=2).transpose(1, 0, 2).reshape(128, 4 * CT)
                 .astype(np.float32),
        "gmat": (np.arange(128)[:, None] // GSIZE ==
                 np.arange(8)[None, :]).astype(np.float32),
        "gmat_t": np.ascontiguousarray(
            (np.arange(128)[:, None] // GSIZE ==
             np.arange(8)[None, :]).astype(np.float32).T),
    }

    in_maps = []
    for i in range(NCORES):
        b, h = i // 2, i % 2
        x2 = x[b].reshape(C, N)
        if h == 0:
            x_cn = _round_f32r(x2)
        else:
            x_cn = _round_f32r(
                np.concatenate([x2[:, NQ:], x2[:, :NQ]], axis=1))
        xb = np.ascontiguousarray(x2.T[h * NQ:(h + 1) * NQ] + b_proj[None, :])
        m = {"x_cn": x_cn, "xb_t": xb}
        m.update(shared)
        in_maps.append(m)

    nc = _get_prog()
    trace = os.environ.get("KERNEL_TRACE", "0") == "1"
    try:
        res = run_bass_kernel_spmd(nc, in_maps, list(range(NCORES)),
                                   trace=trace)
    except Exception:
        # transient NRT failures (e.g. a wedged core) usually clear on retry
        import time
        time.sleep(5)
        res = run_bass_kernel_spmd(nc, in_maps, list(range(NCORES)),
                                   trace=trace)
    if trace:
        kernel.last_exec_time_ns = res.exec_time_ns
        kernel.last_results = res

    out = np.empty((B, C, N), dtype=np.float32)
    for i in range(NCORES):
        b, h = i // 2, i % 2
        out[b][:, h * NQ:(h + 1) * NQ] = res.results[i]["y_t"].T
    return out.reshape(B, C, HH, WW)


# revision 29
# speedup vs baseline: 1.0367x; 1.0367x over previous
"""Trainium2 Bass kernel for nn_AttentionBlock (GroupNorm + single-head self-attention + proj + residual).

Input  x [4, 512, 64, 64] f32.  8 NeuronCores: core i handles batch b=i//2,
query-half h=i%2 (2048 of the 4096 spatial positions).  Each core computes
GroupNorm + full K/V for its batch element, Q only for its half, flash-style
attention over all 4096 keys, the output projection and residual for its half.
No collectives: host shards inputs / gathers outputs.

Layouts (per core), n=4096, nq=2048, c=512:
  hf  = GN(x)      [c, n]    (4 SBUF tiles [128, 4096], in-place over x)
  k   = Wk hf + bk [c, n]    (resident SBUF, 4 tiles [128, 4096])
  q   = Wq hf + bq [c, nq]   (spilled to DRAM, streamed back per q-chunk)
  v^T = (Wv hf + bv)^T [n, c] (spilled to DRAM, streamed back per (qc, j))
  S^T chunk [j:128, q:512] = k_chunk^T q_chunk   (PSUM)
  E = exp(S^T * 1/sqrt(c))  -> O[co,q] += v^T_j[:,co]^T E ;  denom[1,q] += 1^T E
  y^T [q:128, c:512] = (O chunk)^T Wp^T ; y = y^T * (1/denom) + (x^T + b_proj)
"""

import os
import numpy as np

B, C, HH, WW = 4, 512, 64, 64
N = HH * WW            # 4096
NQ = N // 2            # 2048 queries per core
NCORES = 8
CT = C // 128          # 4 channel tiles
PT = N // 512          # 8 spatial chunks of 512
QT = NQ // 512         # 4 query chunks of 512
JT = N // 128          # 32 key chunks of 128
GSIZE = 16             # channels per group
EPS = 1e-5
SCALE = 1.0 / float(np.sqrt(C))

MM_DT_NAME = os.environ.get("KERNEL_MM_DT", "float32r")

_PROG = None
_PROG_DT = None


def _build_program(mm_dt_name):
    import concourse.bacc as bacc
    import concourse.tile as tile
    from concourse import mybir
    from contextlib import ExitStack

    F32 = mybir.dt.float32
    MM = getattr(mybir.dt, mm_dt_name)

    nc = bacc.Bacc("TRN2", target_bir_lowering=False, debug=False,
                   num_devices=NCORES)

    def din(name, shape, dt=None):
        return nc.dram_tensor(name, shape, dt or F32, kind="ExternalInput").ap()

    x_cn = din("x_cn", [C, N], MM)      # x for this batch, query-half first
    xb_t = din("xb_t", [NQ, C])         # x^T residual slice + b_proj
    w_qT = din("w_qT", [C, C], MM)
    w_kT = din("w_kT", [C, C], MM)
    w_vT = din("w_vT", [C, C], MM)
    w_pT = din("w_pT", [C, C], MM)
    b_v = din("b_v", [1, C], MM)
    cols = din("cols", [128, 4 * CT])   # per c-tile: gamma, beta, b_q, b_k
    gmat = din("gmat", [128, 8])        # group membership (p//16 == u)
    gmat_t = din("gmat_t", [8, 128])
    y_t = nc.dram_tensor("y_t", [NQ, C], F32, kind="ExternalOutput").ap()

    AF = mybir.ActivationFunctionType
    OP = mybir.AluOpType

    with tile.TileContext(nc) as tc, ExitStack() as ctx:
        persist = ctx.enter_context(tc.tile_pool(name="persist", bufs=1))
        kpool = ctx.enter_context(tc.tile_pool(name="kpool", bufs=1))
        qa0pool = ctx.enter_context(tc.tile_pool(name="qa0", bufs=1))
        dram = ctx.enter_context(tc.tile_pool(name="dram", bufs=1, space="DRAM"))

        # ---- persistent small constants ----
        gma = persist.tile([128, 8], F32)
        nc.sync.dma_start(out=gma, in_=gmat)
        gmt = persist.tile([8, 128], F32)
        nc.sync.dma_start(out=gmt, in_=gmat_t)
        one1 = persist.tile([1, 1], F32)
        nc.vector.memset(one1, 1.0)
        ones_row = persist.tile([1, 128], MM)
        ones_col = persist.tile([128, 1], MM)
        if MM is F32:
            nc.vector.memset(ones_row, 1.0)
            nc.vector.memset(ones_col, 1.0)
        else:
            ones_st_r = persist.tile([1, 128], F32)
            nc.vector.memset(ones_st_r, 1.0)
            nc.vector.tensor_copy(ones_row, ones_st_r)
            ones_st_c = persist.tile([128, 1], F32)
            nc.vector.memset(ones_st_c, 1.0)
            nc.vector.tensor_copy(ones_col, ones_st_c)
        eps8 = persist.tile([8, 1], F32)
        nc.vector.memset(eps8, EPS)
        warm_a = persist.tile([128, 128], mybir.dt.bfloat16)
        nc.vector.memset(warm_a, 0.03)

        def emit_burst(wpool, wppool, dep_ap, n, nm, pstag="warmps"):
            # Dense bf16 matmuls (normal PE mode) chained to dep_ap: trips the
            # PE activity monitor into the fast-clock state.  fp32r matmuls
            # alone don't reliably do this.
            wb = wpool.tile([128, 512], mybir.dt.bfloat16, tag="warmb",
                            name=f"wb_{nm}")
            nc.vector.tensor_copy(wb, dep_ap)
            wps = wppool.tile([128, 512], F32, tag=pstag,
                              name=f"wps_{nm}", bufs=2)
            for wi in range(n):
                nc.tensor.matmul(wps, warm_a, wb,
                                 start=(wi == 0), stop=(wi == n - 1))
        cols_t = persist.tile([128, 4 * CT], F32)
        nc.sync.dma_start(out=cols_t, in_=cols)
        gcol = [cols_t[:, 4 * t:4 * t + 1] for t in range(CT)]
        bcol = [cols_t[:, 4 * t + 1:4 * t + 2] for t in range(CT)]
        bqcol = [cols_t[:, 4 * t + 2:4 * t + 3] for t in range(CT)]
        bkcol = [cols_t[:, 4 * t + 3:4 * t + 4] for t in range(CT)]
        # b_v row (already MM dtype in DRAM)
        bvr = persist.tile([1, C], MM)
        nc.sync.dma_start(out=bvr, in_=b_v)
        # w_proj^T tiles (persist: used at the end of every q-chunk)
        wp_big = persist.tile([128, CT, C], MM)
        nc.sync.dma_start(out=wp_big,
                          in_=w_pT.rearrange("(t p) o -> p t o", t=CT))
        wp = [wp_big[:, t, :] for t in range(CT)]

        # k stays resident for the whole kernel
        k_tiles = [kpool.tile([128, N], MM, name=f"k_{t}", tag=f"k{t}")
                   for t in range(CT)]
        # first 512 query columns stay resident: attention qc=0 starts without
        # waiting for the q spill/reload round-trip
        qa0_tiles = [qa0pool.tile([128, 512], MM, name=f"qa0_{t}",
                                  tag=f"qa0{t}") for t in range(CT)]
        # spill targets
        vt_dram = dram.tile([JT, 128, C], MM)     # v^T as 32 j-tiles [128, 512]
        q_dram = dram.tile([CT, 128, NQ], MM)     # q in [c, nq] layout

        # GroupNorm is folded into the qkv weights:  hf = x*sc + bc  =>
        #   q/k/v = (W ∘ sc) x + (W bc + b_qkv).
        # x arrives pre-rounded to the matmul dtype, so it feeds the matmuls
        # directly; no hf tiles and no big apply pass.  (Assumes gamma has no
        # exact zeros: the weight-bias matvec uses bc/sc on the scaled W.)
        with tc.tile_pool(name="xpool", bufs=1) as xpool, \
             tc.tile_pool(name="wmat", bufs=1) as wmat, \
             tc.tile_pool(name="qkvsb", bufs=3) as qkvsb, \
             tc.tile_pool(name="gnsb", bufs=2) as gnsb, \
             tc.tile_pool(name="qps", bufs=1, space="PSUM") as qps:

            x_tiles = [xpool.tile([128, N], MM, name=f"x_{t}", tag=f"x{t}")
                       for t in range(CT)]
            from concourse.bass import _add_dep_helper
            prev_dma = None
            for t in range(CT):
                dma_i = nc.sync.dma_start(out=x_tiles[t],
                                          in_=x_cn[t * 128:(t + 1) * 128, :])
                if prev_dma is not None:
                    _add_dep_helper(dma_i.ins, prev_dma.ins, sync=True,
                                    reason="serialize x tile loads")
                prev_dma = dma_i

            def load_w(srcw, nm):
                w_big = wmat.tile([128, CT, C], MM, name=f"{nm}_big",
                                  tag="w", bufs=2)
                nc.sync.dma_start(
                    out=w_big,
                    in_=srcw.rearrange("(t p) o -> p t o", t=CT))
                return [w_big[:, t, :] for t in range(CT)]

            wv = load_w(w_vT, "wv")

            # ---------------- GroupNorm statistics ----------------
            sc_f = []
            bct = []
            wb_init = gnsb.tile([128, 512], mybir.dt.bfloat16, tag="warmb")
            nc.vector.memset(wb_init, 0.01)
            wps_init = qps.tile([128, 512], F32, tag="g", bufs=2)
            for wi in range(25):
                nc.tensor.matmul(wps_init, warm_a, wb_init,
                                 start=(wi == 0), stop=(wi == 24))
            BRIDGE = (30, 25, 20, 15)
            for t in range(CT):
                x_f = (x_tiles[t] if MM is F32 else x_tiles[t].bitcast(F32))
                # bf16 junk matmuls chained to this x tile: keep the PE busy
                # (and its activity monitor in the fast-clock state) while the
                # GroupNorm statistics for later tiles are still in flight
                emit_burst(gnsb, qps, x_tiles[t][:, 0:512], BRIDGE[t],
                           f"br{t}", pstag="g")
                # st2 = [sum(x), sum(x^2)] per channel: plain sum on DVE,
                # squared sum via ACT Square + accumulate (k tile is scratch)
                st2 = gnsb.tile([128, 2], F32, tag="st2")
                nc.vector.reduce_sum(out=st2[:, 0:1], in_=x_f,
                                     axis=mybir.AxisListType.X)
                nc.scalar.activation(out=k_tiles[t], in_=x_f, func=AF.Square,
                                     accum_out=st2[:, 1:2])
                # group-aggregate: [8, 2] sums over the 16 channels per group
                gps = qps.tile([8, 2], F32, tag="g", bufs=2)
                nc.tensor.matmul(gps, gma, st2, start=True, stop=True)
                grp = gnsb.tile([8, 2], F32, tag="grp")
                nc.scalar.mul(out=grp, in_=gps, mul=1.0 / (GSIZE * N))
                gm2 = gnsb.tile([8, 1], F32, tag="gm2")
                nc.vector.tensor_mul(gm2, grp[:, 0:1], grp[:, 0:1])
                var = gnsb.tile([8, 1], F32, tag="var")
                nc.vector.tensor_sub(var, grp[:, 1:2], gm2)
                std = gnsb.tile([8, 1], F32, tag="std")
                nc.scalar.activation(out=std, in_=var, func=AF.Sqrt,
                                     bias=eps8, scale=1.0)
                gout = gnsb.tile([8, 2], F32, tag="gout")
                nc.vector.tensor_copy(gout[:, 0:1], grp[:, 0:1])
                nc.vector.reciprocal(out=gout[:, 1:2], in_=std)
                # expand group stats back to per-channel [128, 2]
                eps_ps = qps.tile([128, 2], F32, tag="g", bufs=2,
                                  name=f"eps_ps{t}")
                nc.tensor.matmul(eps_ps, gmt, gout, start=True, stop=True)
                pg = gnsb.tile([128, 2], F32, tag="pg")
                nc.scalar.copy(out=pg, in_=eps_ps)
                # per-channel scale = gamma*rstd ; bias = beta - mean*scale
                sc_t = gnsb.tile([128, 1], F32, tag=f"sc{t}", bufs=1)
                nc.vector.tensor_mul(sc_t, gcol[t], pg[:, 1:2])
                sc_f.append(sc_t)
                bc_t = gnsb.tile([128, 1], F32, tag="bc")
                nc.vector.tensor_mul(bc_t, pg[:, 0:1], sc_t)
                nc.vector.tensor_sub(bc_t, bcol[t], bc_t)
                rsc = gnsb.tile([128, 1], F32, tag="rsc")
                nc.vector.reciprocal(out=rsc, in_=sc_t)
                bct_t = gnsb.tile([128, 1], MM, tag=f"bct{t}", bufs=1)
                nc.vector.tensor_mul(bct_t, bc_t, rsc)
                bct.append(bct_t)
                # scale this channel-tile of Wv in place (Wk/Wq later)
                wv_f = (wv[t] if MM is F32 else wv[t].bitcast(F32))
                nc.vector.tensor_scalar_mul(out=wv[t], in0=wv_f,
                                            scalar1=sc_t)

            # weight-bias matvecs:  row_m = sum_c (bc/sc)_c^T (W ∘ sc)_c
            def bias_row(tiles, nm):
                row_ps = qps.tile([1, C], F32, tag="g", bufs=2,
                                  name=f"brow_{nm}")
                for c in range(CT):
                    nc.tensor.matmul(row_ps, bct[c], tiles[c],
                                     start=(c == 0), stop=(c == CT - 1))
                row_sb = gnsb.tile([1, C], F32, tag=f"brs_{nm}", bufs=1)
                nc.scalar.copy(out=row_sb, in_=row_ps)
                return row_sb

            def bias_cols(row_sb, host_cols, nm):
                cols = []
                for o in range(CT):
                    bt_ps = qps.tile([128, 1], F32, tag="g", bufs=2,
                                     name=f"bt_{nm}{o}")
                    nc.tensor.transpose(bt_ps,
                                        row_sb[0:1, o * 128:(o + 1) * 128],
                                        one1)
                    tot = gnsb.tile([128, 1], F32, tag=f"btot_{nm}{o}",
                                    bufs=1)
                    nc.vector.tensor_add(tot, bt_ps, host_cols[o])
                    cols.append(tot)
                return cols

            def scale_w(tiles):
                for t in range(CT):
                    m_f = (tiles[t] if MM is F32 else tiles[t].bitcast(F32))
                    nc.vector.tensor_scalar_mul(out=tiles[t], in0=m_f,
                                                scalar1=sc_f[t])

            vrow = bias_row(wv, "v")
            bvr_tot = gnsb.tile([1, C], MM, tag="bvrt", bufs=1)
            nc.vector.tensor_add(bvr_tot, vrow,
                                 (bvr if MM is F32 else bvr.bitcast(F32)))
            # k weights: second buffer slot is free, so this overlaps v-phase
            wk_l = load_w(w_kT, "wk")
            scale_w(wk_l)
            bk_tot = bias_cols(bias_row(wk_l, "k"), bkcol, "k")

            # ---------------- QKV ----------------
            # v^T = x^T Wv' + bv' : 32 tiles [128p, 512c] -> DRAM
            for p in range(JT):
                vt_ps = qps.tile([128, C], F32, tag="mm", bufs=4)
                for c in range(CT):
                    nc.tensor.matmul(vt_ps,
                                     x_tiles[c][:, p * 128:(p + 1) * 128],
                                     wv[c], start=(c == 0), stop=False)
                nc.tensor.matmul(vt_ps, ones_row, bvr_tot,
                                 start=False, stop=True)
                vt_sb = qkvsb.tile([128, C], MM, tag="vt")
                nc.scalar.copy(out=vt_sb, in_=vt_ps)
                nc.sync.dma_start(out=vt_dram[p], in_=vt_sb)
                if p in (10, 21):
                    emit_burst(qkvsb, qps, vt_sb, 15, f"v{p}", pstag="g")

            # q weights reuse Wv's buffer slots (released by the v-phase)
            wq_l = load_w(w_qT, "wq")
            scale_w(wq_l)
            bq_tot = bias_cols(bias_row(wq_l, "q"), bqcol, "q")

            # k = Wk'^T x + bk' : resident [c,n] tiles
            for o in range(CT):
                for p in range(PT):
                    k_ps = qps.tile([128, 512], F32, tag="mm", bufs=4)
                    for c in range(CT):
                        nc.tensor.matmul(k_ps,
                                         wk_l[c][:, o * 128:(o + 1) * 128],
                                         x_tiles[c][:, p * 512:(p + 1) * 512],
                                         start=(c == 0), stop=(c == CT - 1))
                    nc.vector.tensor_scalar_add(
                        out=k_tiles[o][:, p * 512:(p + 1) * 512],
                        in0=k_ps, scalar1=bk_tot[o])
                    if (o, p) in ((0, 0), (2, 0)):
                        emit_burst(qkvsb, qps,
                                   k_tiles[o][:, p * 512:(p + 1) * 512],
                                   15, f"k{o}", pstag="g")

            # q = Wq'^T x + bq' for first NQ columns; p-major so the first
            # 512 query columns are ready first (they stay resident in SBUF)
            for p in range(QT):
                for o in range(CT):
                    q_ps = qps.tile([128, 512], F32, tag="mm", bufs=4)
                    for c in range(CT):
                        nc.tensor.matmul(q_ps,
                                         wq_l[c][:, o * 128:(o + 1) * 128],
                                         x_tiles[c][:, p * 512:(p + 1) * 512],
                                         start=(c == 0), stop=(c == CT - 1))
                    if p == 0:
                        nc.vector.tensor_scalar_add(out=qa0_tiles[o],
                                                    in0=q_ps,
                                                    scalar1=bq_tot[o])
                        if o == 0:
                            emit_burst(qkvsb, qps, qa0_tiles[0], 15,
                                       "qst", pstag="g")
                    else:
                        q_sb = qkvsb.tile([128, 512], MM, tag="q")
                        nc.vector.tensor_scalar_add(out=q_sb, in0=q_ps,
                                                    scalar1=bq_tot[o])
                        nc.sync.dma_start(
                            out=q_dram[o][:, p * 512:(p + 1) * 512],
                            in_=q_sb)

        # ---------------- attention + proj (per 512-wide q-chunk) ----------------
        with tc.tile_pool(name="qa", bufs=2) as qapool, \
             tc.tile_pool(name="estream", bufs=3) as epool, \
             tc.tile_pool(name="vstream", bufs=4) as vpool, \
             tc.tile_pool(name="osb", bufs=2) as opool, \
             tc.tile_pool(name="ysb", bufs=2) as ypool, \
             tc.tile_pool(name="xbst", bufs=3) as xbpool, \
             tc.tile_pool(name="dsb", bufs=2) as dpool, \
             tc.tile_pool(name="psS", bufs=2, space="PSUM") as psS, \
             tc.tile_pool(name="psO", bufs=1, space="PSUM") as psO, \
             tc.tile_pool(name="psD", bufs=1, space="PSUM") as psD, \
             tc.tile_pool(name="psY", bufs=1, space="PSUM") as psY:

            for qc in range(QT):
                if qc == 0:
                    qa = qa0_tiles
                else:
                    # stream this q-chunk back: qa[c] = q[c, qc*512 : +512]
                    qa = []
                    for c in range(CT):
                        qa_c = qapool.tile([128, 512], MM, name=f"qa_{c}",
                                           tag=f"qa{c}")
                        nc.sync.dma_start(
                            out=qa_c,
                            in_=q_dram[c][:, qc * 512:(qc + 1) * 512])
                        qa.append(qa_c)

                o_ps = [psO.tile([128, 512], F32, name=f"o_ps{co}",
                                 tag=f"o{co}") for co in range(CT)]
                d_ps = psD.tile([1, 512], F32, tag="d")

                def s_exp_v(j):
                    # S^T chunk + exp, and prefetch v^T tile for this j
                    vt_sb = vpool.tile([128, C], MM, tag="vt2")
                    nc.sync.dma_start(out=vt_sb, in_=vt_dram[j])
                    s_ps = psS.tile([128, 512], F32, tag="s")
                    for c in range(CT):
                        nc.tensor.matmul(s_ps,
                                         k_tiles[c][:, j * 128:(j + 1) * 128],
                                         qa[c], start=(c == 0), stop=(c == CT - 1))
                    e_sb = epool.tile([128, 512], MM, tag="e")
                    nc.scalar.activation(out=e_sb, in_=s_ps, func=AF.Exp,
                                         scale=SCALE)
                    return e_sb, vt_sb

                e_cur, v_cur = s_exp_v(0)
                for j in range(JT):
                    nxt = s_exp_v(j + 1) if j + 1 < JT else None
                    first, last = (j == 0), (j == JT - 1)
                    for co in range(CT):
                        nc.tensor.matmul(o_ps[co],
                                         v_cur[:, co * 128:(co + 1) * 128],
                                         e_cur, start=first, stop=last)
                    nc.tensor.matmul(d_ps, ones_col, e_cur,
                                     start=first, stop=last)
                    if nxt is not None:
                        e_cur, v_cur = nxt

                # denominators -> per-query reciprocal [128,1] per 128-row block
                d_sb = dpool.tile([1, 512], F32, tag="dsb")
                nc.scalar.copy(out=d_sb, in_=d_ps)
                rc = []
                for qs in range(4):
                    dt_ps = psD.tile([128, 1], F32, name=f"dt_ps{qs}", tag="d")
                    nc.tensor.transpose(dt_ps,
                                        d_sb[0:1, qs * 128:(qs + 1) * 128],
                                        one1)
                    rc_t = dpool.tile([128, 1], F32, name=f"rc_{qs}",
                                      tag=f"rc{qs}")
                    nc.vector.reciprocal(out=rc_t, in_=dt_ps)
                    rc.append(rc_t)

                # O -> SBUF (rounds to MM dtype)
                o_sb = []
                for co in range(CT):
                    o_t = opool.tile([128, 512], MM, name=f"o_sb{co}",
                                     tag=f"ob{co}")
                    nc.vector.tensor_copy(o_t, o_ps[co])
                    o_sb.append(o_t)

                # proj + 1/denom + residual, per 128-row output block
                for qs in range(4):
                    y_ps = psY.tile([128, C], F32, name=f"y_ps{qs}",
                                    tag="y")
                    for c in range(CT):
                        nc.tensor.matmul(y_ps,
                                         o_sb[c][:, qs * 128:(qs + 1) * 128],
                                         wp[c], start=(c == 0), stop=(c == CT - 1))
                    row0 = qc * 512 + qs * 128
                    xb_sb = xbpool.tile([128, C], F32, tag="xb")
                    nc.sync.dma_start(out=xb_sb, in_=xb_t[row0:row0 + 128, :])
                    y1 = ypool.tile([128, C], F32, tag="y1")
                    nc.vector.tensor_scalar_mul(out=y1, in0=y_ps, scalar1=rc[qs])
                    yo = ypool.tile([128, C], F32, tag="yo")
                    nc.vector.tensor_add(yo, y1, xb_sb)
                    nc.sync.dma_start(out=y_t[row0:row0 + 128, :], in_=yo)

    nc.compile()
    return nc


def _get_prog():
    global _PROG, _PROG_DT
    if _PROG is None or _PROG_DT != MM_DT_NAME:
        _PROG = _build_program(MM_DT_NAME)
        _PROG_DT = MM_DT_NAME
    return _PROG


def _round_f32r(a):
    """RNE to 11 explicit mantissa bits (the fp32r matmul input format)."""
    if MM_DT_NAME != "float32r":
        return a
    b = np.ascontiguousarray(a, dtype=np.float32).view(np.uint32)
    shift = 12
    lsb = (b >> shift) & 1
    mask = np.uint32((~((1 << shift) - 1)) & 0xFFFFFFFF)
    out = (b + np.uint32((1 << (shift - 1)) - 1) + lsb) & mask
    return out.view(np.float32)


def kernel(x, gamma, beta, w_qkv, b_qkv, w_proj, b_proj):
    from concourse.bass_utils import run_bass_kernel_spmd

    x = np.asarray(x, dtype=np.float32)
    gamma = np.asarray(gamma, dtype=np.float32)
    beta = np.asarray(beta, dtype=np.float32)
    w_qkv = np.asarray(w_qkv, dtype=np.float32)
    b_qkv = np.asarray(b_qkv, dtype=np.float32)
    w_proj = np.asarray(w_proj, dtype=np.float32)
    b_proj = np.asarray(b_proj, dtype=np.float32)

    shared = {
        "w_qT": _round_f32r(w_qkv[0:C].T),
        "w_kT": _round_f32r(w_qkv[C:2 * C].T),
        "w_vT": _round_f32r(w_qkv[2 * C:3 * C].T),
        "w_pT": _round_f32r(w_proj.T),
        "b_v": _round_f32r(b_qkv[2 * C:3 * C].reshape(1, C)),
        "cols": np.stack([gamma.reshape(CT, 128),
                          beta.reshape(CT, 128),
                          b_qkv[0:C].reshape(CT, 128),
                          b_qkv[C:2 * C].reshape(CT, 128)],
                         axis=2).transpose(1, 0, 2).reshape(128, 4 * CT)
                 .astype(np.float32),
        "gmat": (np.arange(128)[:, None] // GSIZE ==
                 np.arange(8)[None, :]).astype(np.float32),
        "gmat_t": np.ascontiguousarray(
            (np.arange(128)[:, None] // GSIZE ==
             np.arange(8)[None, :]).astype(np.float32).T),
    }

    in_maps = []
    for i in range(NCORES):
        b, h = i // 2, i % 2
        x2 = x[b].reshape(C, N)
        if h == 0:
            x_cn = _round_f32r(x2)
        else:
            x_cn = _round_f32r(
                np.concatenate([x2[:, NQ:], x2[:, :NQ]], axis=1))
        xb = np.ascontiguousarray(x2.T[h * NQ:(h + 1) * NQ] + b_proj[None, :])
        m = {"x_cn": x_cn, "xb_t": xb}
        m.update(shared)
        in_maps.append(m)

    nc = _get_prog()
    trace = os.environ.get("KERNEL_TRACE", "0") == "1"
    try:
        res = run_bass_kernel_spmd(nc, in_maps, list(range(NCORES)),
                                   trace=trace)
    except Exception:
        # transient NRT failures (e.g. a wedged core) usually clear on retry
        import time
        time.sleep(5)
        res = run_bass_kernel_spmd(nc, in_maps, list(range(NCORES)),
                                   trace=trace)
    if trace:
        kernel.last_exec_time_ns = res.exec_time_ns
        kernel.last_results = res

    out = np.empty((B, C, N), dtype=np.float32)
    for i in range(NCORES):
        b, h = i // 2, i % 2
        out[b][:, h * NQ:(h + 1) * NQ] = res.results[i]["y_t"].T
    return out.reshape(B, C, HH, WW)
